# revision 18
# baseline (speedup 1.0000x reference)
"""Trainium2 Bass kernel for nn_IntegratedLaughterModel.

Strategy (pure data parallel, 8 samples/core):
  - Algebraic reduction: scores[b,h,s] = x[b,s,:] @ qk[:,h] where
    qk = (Wk reshaped) @ q_tom / sqrt(DH)  (host-precomputed [D, NH]).
    This removes the two [S,D]x[D,D] matmuls entirely.
  - Single pass over x per core: per 512-token group,
      PE-transpose x chunks -> scores matmul (fp32r) -> mask via rank-4
      log-mask matmul -> ACT exp (with accum Z) -> PE-transpose weights ->
      pooling matmul accumulating [11, D] per sample.
    rows 0..7  = unnormalized attn-weighted sums per head,
    rows 8..10 = masked means (mean/setup/punch; masks pre-scaled on host,
                 folded in as exp(log(mask)) rows so one pooling matmul).
  - Small per-core head (feature-major, [128d, 8b] tiles) computes the
    ToM/GCACU/CLoST/mHC/SEVADE/final MLPs on-device.
"""

import os
import numpy as np

B, S, D, HID, NH = 64, 2048, 512, 512, 8
DH = D // NH
NCORES = 8
BPC = B // NCORES   # samples per core
NG = 4              # 512-token groups per sample
GT = 512            # tokens per group
NT = 4              # 128-token subtiles per group
NCD = 4             # d-chunks of 128
EPS = 1e-4
MASK_NEG = -30.0    # additive penalty for masked tokens (exp(-30) ~ 1e-13)
LOG_FLOOR = -80.0

F32R = os.environ.get("KERNEL_F32R", "1") == "1"

_CACHE = {}
LAST_RESULT = None


def _build_program():
    import concourse.bacc as bacc
    import concourse.tile as tile
    from concourse import mybir
    from contextlib import ExitStack

    f32 = mybir.dt.float32
    bf16 = mybir.dt.bfloat16
    AF = mybir.ActivationFunctionType
    ALU = mybir.AluOpType

    nc = bacc.Bacc("TRN2", target_bir_lowering=False, debug=False,
                   enable_asserts=False)

    # ---- DRAM I/O ----
    xg_d = nc.dram_tensor("xg", [BPC, 128, NG * NT * GT], bf16,
                          kind="ExternalInput").ap()
    xtg_d = nc.dram_tensor("xtg", [BPC, 128, NG * NCD * GT], bf16,
                           kind="ExternalInput").ap()
    pen_d = nc.dram_tensor("pen", [1, BPC * NG * GT], bf16, kind="ExternalInput").ap()
    qk_d = nc.dram_tensor("qk", [128, NCD * 16], bf16, kind="ExternalInput").ap()
    maskt_d = nc.dram_tensor("maskt", [128, BPC * NG * 12], bf16,
                             kind="ExternalInput").ap()
    invcnt_d = nc.dram_tensor("invcnt", [128, BPC // 2], f32,
                              kind="ExternalInput").ap()
    id_d = nc.dram_tensor("ident", [128, 128], bf16, kind="ExternalInput").ap()
    id32_d = nc.dram_tensor("ident32", [16, 16], f32, kind="ExternalInput").ap()
    wv_d = nc.dram_tensor("wv", [128, 2048], bf16, kind="ExternalInput").ap()
    wtf_d = nc.dram_tensor("wtf", [128, 2048], bf16, kind="ExternalInput").ap()
    wg1_d = nc.dram_tensor("wg1", [128, 2048], bf16, kind="ExternalInput").ap()
    wg2_d = nc.dram_tensor("wg2", [128, 2048], bf16, kind="ExternalInput").ap()
    wc1_d = nc.dram_tensor("wc1", [128, 4096], bf16, kind="ExternalInput").ap()
    ws1_d = nc.dram_tensor("ws1", [128, 2048], bf16, kind="ExternalInput").ap()
    ws1t_d = nc.dram_tensor("ws1t", [3, 512], bf16, kind="ExternalInput").ap()
    wf1_d = nc.dram_tensor("wf1", [128, 2048], bf16, kind="ExternalInput").ap()
    wf1t_d = nc.dram_tensor("wf1t", [3, 512], bf16, kind="ExternalInput").ap()
    vecs_d = nc.dram_tensor("vecs", [128, 20], bf16, kind="ExternalInput").ap()
    bvecs_d = nc.dram_tensor("bvecs", [128, 20], f32, kind="ExternalInput").ap()
    b5_d = nc.dram_tensor("b5", [1, 5], f32, kind="ExternalInput").ap()
    m3_d = nc.dram_tensor("m3", [1, 9], f32, kind="ExternalInput").ap()
    out_d = nc.dram_tensor("out", [1, BPC], f32, kind="ExternalOutput").ap()
    diag_d = nc.dram_tensor("diag", [BPC * 11, D], f32, kind="ExternalOutput").ap()

    with tile.TileContext(nc) as tc, ExitStack() as ctx:
        cst = ctx.enter_context(tc.tile_pool(name="cst", bufs=1))

        def static(name, shape, src, dt=f32):
            t = cst.tile(shape, dt, tag=name, name=name)
            nc.sync.dma_start(out=t[:], in_=src)
            return t

        def static_g2(name, shape, src, dt=f32):
            t = cst.tile(shape, dt, tag=name, name=name)
            nc.gpsimd.dma_start(out=t[:], in_=src)
            return t

        ones_sb = cst.tile([128, 1], f32, tag="ones")
        nc.vector.memset(ones_sb[:], 1.0)
        ones16_sb = cst.tile([128, 1], bf16, tag="ones16")
        nc.vector.memset(ones16_sb[:], 1.0)

        # pooledT: [128 d, c-chunk x sample x quantity] feature-major pooled
        pTall = cst.tile([128, NCD * BPC * 11], bf16, tag="pTall", name="pTall")

        H = {}

        def static_g(name, shape, src_ap, dt=f32):
            t = cst.tile(shape, dt, tag=name, name=name)
            nc.gpsimd.dma_start(out=t[:], in_=src_ap)
            return t

        def load_head_weights(tranche):
            if tranche == 0:
                H["wv"] = static_g("wv", [128, 2048], wv_d, bf16)
                H["wtf"] = static_g("wtf", [128, 2048], wtf_d, bf16)
                H["vecs"] = static_g("vecs", [128, 20], vecs_d, bf16)
                H["bvecs"] = static_g("bvecs", [128, 20], bvecs_d)
                H["b5"] = static_g("b5", [1, 5], b5_d)
                H["m3"] = static_g("m3", [1, 9], m3_d)
            elif tranche == 1:
                H["wg1"] = static_g("wg1", [128, 2048], wg1_d, bf16)
                H["wg2"] = static_g("wg2", [128, 2048], wg2_d, bf16)
                H["wc1"] = static_g("wc1", [128, 4096], wc1_d, bf16)
            else:
                H["ws1"] = static_g("ws1", [128, 2048], ws1_d, bf16)
                H["ws1t"] = static_g("ws1t", [3, 512], ws1t_d, bf16)
                H["wf1"] = static_g("wf1", [128, 2048], wf1_d, bf16)
                H["wf1t"] = static_g("wf1t", [3, 512], wf1t_d, bf16)

        # ================= main streaming pass =================
        with ExitStack() as pctx:
            xg_p = pctx.enter_context(tc.tile_pool(name="xg", bufs=4))
            xtg_p = pctx.enter_context(tc.tile_pool(name="xtg", bufs=4))
            sc_ps_p = pctx.enter_context(tc.tile_pool(name="scps", bufs=2, space="PSUM"))
            w_sb_p = pctx.enter_context(tc.tile_pool(name="wsb", bufs=2))
            wt_ps_p = pctx.enter_context(tc.tile_pool(name="wtps", bufs=2, space="PSUM"))
            wt_sb_p = pctx.enter_context(tc.tile_pool(name="wtsb", bufs=8))
            pool_ps_p = pctx.enter_context(tc.tile_pool(name="poolps", bufs=2, space="PSUM"))
            small_p = pctx.enter_context(tc.tile_pool(name="small", bufs=4))

            qk_sb = static("qk", [128, NCD * 16], qk_d, bf16)
            id_sb = static("ident", [128, 128], id_d, bf16)
            id32_sb = static("ident32", [16, 16], id32_d, f32)
            pen_sb = static("pen", [1, BPC * NG * GT], pen_d, bf16)
            maskt_sb = static("maskt", [128, BPC * NG * 12], maskt_d, bf16)
            invcnt_sb = static("invcnt", [128, BPC // 2], invcnt_d, f32)
            one16 = cst.tile([1, 16], bf16, tag="one16")
            nc.vector.memset(one16[:], 1.0)

            xcache = {}

            def fetch_sample(b):
                if b not in xcache:
                    xt = xtg_p.tile([128, NG * NCD * GT], bf16, tag="xtg",
                                    name=f"xtg{b}")
                    nc.sync.dma_start(out=xt[:], in_=xtg_d[b, :, :])
                    xg = xg_p.tile([128, NG * NT * GT], bf16, tag="xg",
                                   name=f"xg{b}")
                    nc.sync.dma_start(out=xg[:], in_=xg_d[b, :, :])
                    xcache[b] = (xt, xg)
                return xcache[b]

            def emit_pair_group(p, g, pool_pr, zcq):
                """Group g for samples 2p, 2p+1."""
                pair = [fetch_sample(2 * p + j) for j in range(2)]

                sc = sc_ps_p.tile([48, GT], f32, tag="sc", name=f"sc{p}_{g}")
                for j in range(2):
                    xt_s = pair[j][0]
                    for c in range(NCD):
                        nc.tensor.matmul(
                            sc[32 * j:32 * j + 16, :],
                            qk_sb[:, c * 16:c * 16 + 16],
                            xt_s[:, (g * NCD + c) * GT:(g * NCD + c + 1) * GT],
                            start=(c == 0), stop=False)
                    col0 = ((2 * p + j) * NG + g) * GT
                    nc.tensor.matmul(sc[32 * j:32 * j + 16, :], one16[:],
                                     pen_sb[:, col0:col0 + GT],
                                     start=False, stop=True)

                w_sb = w_sb_p.tile([48, GT], bf16, tag="w", name=f"w{p}_{g}")
                nc.scalar.activation(w_sb[:], sc[:], AF.Exp,
                                     accum_out=zcq[:, g:g + 1])

                wt_ps = wt_ps_p.tile([128, 64], f32, tag="wt", name=f"wtp{p}_{g}")
                wtb = wt_ps[:].bitcast(bf16)  # [128, 128]
                for j in range(2):
                    for t in range(NT):
                        nc.tensor.transpose(
                            wtb[:, (j * 4 + t) * 16:(j * 4 + t) * 16 + 16],
                            w_sb[32 * j:32 * j + 16, t * 128:(t + 1) * 128],
                            id_sb[32 * j:32 * j + 16, 32 * j:32 * j + 16])

                for j in range(2):
                    b = 2 * p + j
                    wt_t = wt_sb_p.tile([128, 64], bf16, tag="wt",
                                        name=f"wt{b}_{g}")
                    wv3 = wt_t[:].rearrange("p (t v) -> p t v", t=4)
                    nc.vector.tensor_copy(
                        wv3[:, :, 0:8],
                        wtb[:, j * 64:j * 64 + 64].rearrange(
                            "p (t v) -> p t v", t=4)[:, :, 0:8])
                    mcol = (b * NG + g) * 12
                    nc.vector.tensor_copy(
                        wv3[:, :, 8:11],
                        maskt_sb[:, mcol:mcol + 12].rearrange(
                            "p (t v) -> p t v", t=4))
                    xg_s = pair[j][1]
                    for t in range(NT):
                        nc.tensor.matmul(
                            pool_pr[32 * j:32 * j + 16, :],
                            wt_t[:, t * 16:t * 16 + 16],
                            xg_s[:, (g * NT + t) * GT:(g * NT + t + 1) * GT],
                            start=(g == 0 and t == 0),
                            stop=(g == NG - 1 and t == NT - 1))
                if g == NG - 1:
                    xcache.pop(2 * p, None)
                    xcache.pop(2 * p + 1, None)

            def emit_tail(p, pool_pr, zcq):
                zsum = small_p.tile([48, 1], f32, tag="zs", name=f"zs{p}")
                nc.vector.tensor_reduce(zsum[:], zcq[:],
                                        mybir.AxisListType.X, ALU.add)
                zrq = small_p.tile([128, 1], f32, tag="zr", name=f"zr{p}")
                nc.vector.tensor_copy(zrq[:], invcnt_sb[:, p:p + 1])
                for j in range(2):
                    nc.vector.reciprocal(zrq[32 * j:32 * j + 8, :],
                                         zsum[32 * j:32 * j + 8, :])
                for j in range(2):
                    b = 2 * p + j
                    pooled_sb = small_p.tile([11, D], f32, tag="pooled",
                                             name=f"pl{b}")
                    nc.scalar.activation(pooled_sb[:],
                                         pool_pr[32 * j:32 * j + 11, :], AF.Copy,
                                         scale=zrq[32 * j:32 * j + 11, :])
                    if os.environ.get("KERNEL_DIAG", "0") == "1":
                        nc.sync.dma_start(out=diag_d[b * 11:(b + 1) * 11, :],
                                          in_=pooled_sb[:])
                    pt = wt_ps_p.tile([128, 64], f32, tag="wt", name=f"ptp{b}")
                    pt_ps = pt[:, 0:44]
                    for c in range(NCD):
                        nc.tensor.transpose(pt_ps[:, c * 11:(c + 1) * 11],
                                            pooled_sb[:, c * 128:(c + 1) * 128],
                                            id32_sb[:11, :11])
                    dst = pTall[:].rearrange("p (c r) -> p c r", r=BPC * 11)[
                        :, :, b * 11:(b + 1) * 11]
                    nc.vector.tensor_copy(
                        dst, pt_ps.rearrange("p (c v) -> p c v", v=11))

            for p in range(BPC // 2):
                pool_pr = pool_ps_p.tile([48, D], f32, tag="pool",
                                         name=f"pool{p}")
                zcq = small_p.tile([48, NG], f32, tag="zc", name=f"zc{p}")
                for g in range(NG):
                    emit_pair_group(p, g, pool_pr, zcq)
                emit_tail(p, pool_pr, zcq)
                if p < 3:
                    load_head_weights(p)

        # ================= head (feature-major, all 8 samples) =================
        def cview(c, r):
            """[128, 8] view of quantity r across samples in pooledT chunk c."""
            return pTall[:].rearrange("p (c b q) -> p c b q", b=BPC, q=11)[
                :, c, :, r]

        with ExitStack() as hctx:
            pj = hctx.enter_context(tc.tile_pool(name="pj", bufs=5, space="PSUM"))
            ptiny = hctx.enter_context(tc.tile_pool(name="ptiny", bufs=1, space="PSUM"))
            hp = hctx.enter_context(tc.tile_pool(name="hp", bufs=1))
            htmp = hctx.enter_context(tc.tile_pool(name="htmp", bufs=4))

            wv = H["wv"]; wtf = H["wtf"]; wg1 = H["wg1"]; wg2 = H["wg2"]
            wc1 = H["wc1"]; ws1 = H["ws1"]; ws1t = H["ws1t"]; wf1 = H["wf1"]
            wf1t = H["wf1t"]; vecs = H["vecs"]; bvecs = H["bvecs"]
            b5 = H["b5"]; m3 = H["m3"]

            def vcol(k, c):
                return vecs[:, k * 4 + c: k * 4 + c + 1]

            def bcol(k, c):
                return bvecs[:, k * 4 + c: k * 4 + c + 1]

            # ---- fusedT = blockdiag(Wv) applied to attn-pooled heads ----
            fused_sb = []
            for i in range(4):
                ps = pj.tile([128, BPC], f32, tag="proj")
                for hh in range(2):
                    h = 2 * i + hh
                    o = ps[hh * 64:(hh + 1) * 64, :]
                    for c in range(NCD):
                        nc.tensor.matmul(
                            o, wv[:, c * D + h * DH: c * D + (h + 1) * DH],
                            cview(c, h), start=(c == 0), stop=(c == NCD - 1))
                t = hp.tile([128, BPC], bf16, tag=f"fused{i}")
                nc.vector.tensor_copy(t[:], ps[:])
                fused_sb.append(t)

            def proj512(w_tile, rhs_aps, consume, nchunks=4):
                """per jc: ps[j,b] = sum_c W_chunk.T @ rhs_c; consume(jc, ps)."""
                outs = []
                for jc in range(4):
                    ps = pj.tile([128, BPC], f32, tag="proj")
                    for c in range(nchunks):
                        nc.tensor.matmul(
                            ps[:],
                            w_tile[:, c * D + jc * 128: c * D + jc * 128 + 128],
                            rhs_aps[c], start=(c == 0), stop=(c == nchunks - 1))
                    outs.append(consume(jc, ps))
                return outs

            def copy_out(tagp):
                def f(jc, ps):
                    t = hp.tile([128, BPC], bf16, tag=f"{tagp}{jc}")
                    nc.vector.tensor_copy(t[:], ps[:])
                    return t
                return f

            def relu_out(tagp, bk):
                def f(jc, ps):
                    t = hp.tile([128, BPC], bf16, tag=f"{tagp}{jc}")
                    nc.scalar.activation(t[:], ps[:], AF.Relu, bias=bcol(bk, jc))
                    return t
                return f

            # ---- fused_mental; tom_hp pre-sigmoid ----
            fm_sb = proj512(wtf, [t[:] for t in fused_sb], copy_out("fm"))
            s3_ps = ptiny.tile([1, 24], f32, tag="s3")
            for c in range(4):
                nc.tensor.matmul(s3_ps[:, 0:8], vcol(0, c), fm_sb[c][:],
                                 start=(c == 0), stop=(c == 3))

            # ---- GCACU ----
            h1_sb = proj512(wg1, [cview(c, 8) for c in range(4)], relu_out("h1", 0))
            ctx_sb = proj512(wg2, [t[:] for t in h1_sb], copy_out("ctxr"))
            ctxb_sb = []
            for jc in range(4):
                t = hp.tile([128, BPC], bf16, tag=f"ctx{jc}")
                nc.vector.tensor_scalar_add(t[:], ctx_sb[jc][:], bcol(1, jc))
                ctxb_sb.append(t)
            for c in range(4):
                nc.tensor.matmul(s3_ps[:, 8:16], vcol(1, c), ctxb_sb[c][:],
                                 start=(c == 0), stop=(c == 3))

            # ---- CLoST ----
            c1_sb = []
            for jc in range(4):
                ps = pj.tile([128, BPC], f32, tag="proj")
                for cc in range(8):
                    rhs = cview(cc, 9) if cc < 4 else cview(cc - 4, 10)
                    nc.tensor.matmul(
                        ps[:], wc1[:, cc * 512 + jc * 128: cc * 512 + jc * 128 + 128],
                        rhs, start=(cc == 0), stop=(cc == 7))
                t = hp.tile([128, BPC], bf16, tag=f"hc{jc}")
                nc.scalar.activation(t[:], ps[:], AF.Relu, bias=bcol(2, jc))
                c1_sb.append(t)
            for c in range(4):
                nc.tensor.matmul(s3_ps[:, 16:24], vcol(2, c), c1_sb[c][:],
                                 start=(c == 0), stop=(c == 3))
            clost_sb = []
            for c in range(4):
                t = htmp.tile([128, BPC], bf16, tag="cladd")
                nc.vector.tensor_add(t[:], cview(c, 9), cview(c, 10))
                t2 = hp.tile([128, BPC], bf16, tag=f"cl{c}")
                nc.vector.tensor_scalar_mul(t2[:], t[:], 0.5)
                clost_sb.append(t2)

            # ---- scores3: add scalar biases, sigmoid ----
            s3b_sb = hp.tile([1, 24], f32, tag="s3b")
            nc.vector.tensor_scalar_add(s3b_sb[:, 0:8], s3_ps[:, 0:8], b5[:, 0:1])
            nc.vector.tensor_scalar_add(s3b_sb[:, 8:16], s3_ps[:, 8:16], b5[:, 1:2])
            nc.vector.tensor_scalar_add(s3b_sb[:, 16:24], s3_ps[:, 16:24], b5[:, 2:3])
            s3_sb = hp.tile([1, 24], f32, tag="s3s")
            nc.scalar.activation(s3_sb[:], s3b_sb[:], AF.Sigmoid)

            # scores3T [3, 8] via double transpose
            sbt_ps = pj.tile([128, BPC], f32, tag="proj")
            for t in range(3):
                nc.tensor.transpose(sbt_ps[0:8, t:t + 1],
                                    s3_sb[:, t * 8:(t + 1) * 8], id32_sb[:1, :1])
            sbt_sb = hp.tile([8, 3], f32, tag="sbt")
            nc.vector.tensor_copy(sbt_sb[:], sbt_ps[0:8, 0:3])
            s3t_ps = pj.tile([128, BPC], f32, tag="proj")
            nc.tensor.transpose(s3t_ps[0:3, 0:8], sbt_sb[:], id32_sb[:8, :8])
            s3t_sb = hp.tile([3, 8], bf16, tag="s3t")
            nc.vector.tensor_copy(s3t_sb[:], s3t_ps[0:3, 0:8])

            # ---- mHC mix + unit-norm + mean over streams ----
            m3bc = hp.tile([128, 9], f32, tag="m3bc")
            nc.gpsimd.partition_broadcast(m3bc[:], m3[:])
            streams = [fm_sb, ctxb_sb, clost_sb]
            ss_ps = ptiny.tile([1, 24], f32, tag="ss")
            mx = [[None] * 4 for _ in range(3)]
            for i in range(3):
                for c in range(4):
                    a = htmp.tile([128, BPC], bf16, tag="mxa")
                    nc.vector.tensor_scalar_mul(a[:], streams[0][c][:],
                                                m3bc[:, i * 3:i * 3 + 1])
                    bb = htmp.tile([128, BPC], bf16, tag="mxb")
                    nc.vector.scalar_tensor_tensor(
                        bb[:], streams[1][c][:], m3bc[:, i * 3 + 1:i * 3 + 2],
                        a[:], ALU.mult, ALU.add)
                    m_t = hp.tile([128, BPC], bf16, tag=f"mx{i}{c}")
                    nc.vector.scalar_tensor_tensor(
                        m_t[:], streams[2][c][:], m3bc[:, i * 3 + 2:i * 3 + 3],
                        bb[:], ALU.mult, ALU.add)
                    mx[i][c] = m_t
                    sq = htmp.tile([128, BPC], f32, tag="sq")
                    nc.vector.tensor_mul(sq[:], m_t[:], m_t[:])
                    nc.tensor.matmul(ss_ps[:, i * 8:(i + 1) * 8], ones_sb[:],
                                     sq[:], start=(c == 0), stop=(c == 3))
            nrm_sb = hp.tile([1, 24], f32, tag="nrm")
            nc.scalar.activation(nrm_sb[:], ss_ps[:], AF.Sqrt)
            nrm2_sb = hp.tile([1, 24], f32, tag="nrm2")
            nc.vector.tensor_scalar_add(nrm2_sb[:], nrm_sb[:], 1e-6)
            inv_sb = hp.tile([1, 24], f32, tag="inv")
            nc.vector.reciprocal(inv_sb[:], nrm2_sb[:])
            inv3_sb = hp.tile([1, 24], f32, tag="inv3")
            nc.vector.tensor_scalar_mul(inv3_sb[:], inv_sb[:], 1.0 / 3.0)
            inv3b_sb = hp.tile([1, 24], bf16, tag="inv3b")
            nc.vector.tensor_copy(inv3b_sb[:], inv3_sb[:])
            invbc = hp.tile([128, 24], bf16, tag="invbc")
            nc.gpsimd.partition_broadcast(invbc[:], inv3b_sb[:])
            pmix_sb = []
            for c in range(4):
                p0 = htmp.tile([128, BPC], bf16, tag="pm0")
                nc.vector.tensor_mul(p0[:], mx[0][c][:], invbc[:, 0:8])
                p1 = htmp.tile([128, BPC], bf16, tag="pm1")
                nc.vector.tensor_mul(p1[:], mx[1][c][:], invbc[:, 8:16])
                p01 = htmp.tile([128, BPC], bf16, tag="pm01")
                nc.vector.tensor_add(p01[:], p0[:], p1[:])
                p2 = htmp.tile([128, BPC], bf16, tag="pm2")
                nc.vector.tensor_mul(p2[:], mx[2][c][:], invbc[:, 16:24])
                pm = hp.tile([128, BPC], bf16, tag=f"pmix{c}")
                nc.vector.tensor_add(pm[:], p01[:], p2[:])
                pmix_sb.append(pm)

            # ---- SEVADE + final head ----
            fin_ps = ptiny.tile([1, 16], f32, tag="fin")
            for (w_main, w_tail, vk, bk, col) in (
                    (ws1, ws1t, 3, 3, 0), (wf1, wf1t, 4, 4, 8)):
                for jc in range(4):
                    ps = pj.tile([128, BPC], f32, tag="proj")
                    for c in range(4):
                        nc.tensor.matmul(
                            ps[:],
                            w_main[:, c * D + jc * 128: c * D + jc * 128 + 128],
                            pmix_sb[c][:], start=(c == 0), stop=False)
                    nc.tensor.matmul(ps[:], w_tail[:, jc * 128: jc * 128 + 128],
                                     s3t_sb[:], start=False, stop=True)
                    hs = htmp.tile([128, BPC], bf16, tag="hs")
                    nc.scalar.activation(hs[:], ps[:], AF.Relu, bias=bcol(bk, jc))
                    nc.tensor.matmul(fin_ps[:, col:col + 8], vcol(vk, jc), hs[:],
                                     start=(jc == 0), stop=(jc == 3))

            # ---- combine: fin + 0.5*sev + 0.1*safe_logit(mean(s3)) ----
            sev_l = hp.tile([1, 8], f32, tag="sevl")
            nc.vector.tensor_scalar_add(sev_l[:], fin_ps[:, 0:8], b5[:, 3:4])
            fin_l = hp.tile([1, 8], f32, tag="finl")
            nc.vector.tensor_scalar_add(fin_l[:], fin_ps[:, 8:16], b5[:, 4:5])
            t1 = hp.tile([1, 8], f32, tag="t1")
            nc.vector.tensor_add(t1[:], s3_sb[:, 0:8], s3_sb[:, 8:16])
            t2 = hp.tile([1, 8], f32, tag="t2")
            nc.vector.tensor_add(t2[:], t1[:], s3_sb[:, 16:24])
            pm3 = hp.tile([1, 8], f32, tag="pm3")
            nc.vector.tensor_scalar_mul(pm3[:], t2[:], 1.0 / 3.0)
            pcl = hp.tile([1, 8], f32, tag="pcl")
            nc.vector.tensor_scalar(pcl[:], pm3[:], EPS, 1.0 - EPS,
                                    ALU.max, ALU.min)
            lp = hp.tile([1, 8], f32, tag="lp")
            nc.scalar.activation(lp[:], pcl[:], AF.Ln)
            omp = hp.tile([1, 8], f32, tag="omp")
            nc.vector.tensor_scalar(omp[:], pcl[:], -1.0, 1.0, ALU.mult, ALU.add)
            l1p = hp.tile([1, 8], f32, tag="l1p")
            nc.scalar.activation(l1p[:], omp[:], AF.Ln)
            lg = hp.tile([1, 8], f32, tag="lg")
            nc.vector.tensor_sub(lg[:], lp[:], l1p[:])
            o1 = hp.tile([1, 8], f32, tag="o1")
            nc.vector.scalar_tensor_tensor(o1[:], sev_l[:], 0.5, fin_l[:],
                                           ALU.mult, ALU.add)
            o2 = hp.tile([1, 8], f32, tag="o2")
            nc.vector.scalar_tensor_tensor(o2[:], lg[:], 0.1, o1[:],
                                           ALU.mult, ALU.add)
            nc.sync.dma_start(out=out_d[:], in_=o2[:])

    nc.compile()
    return nc


def _pack_w(w, ncol=512):
    w = np.asarray(w, np.float32)
    nchunk = w.shape[0] // 128
    return np.ascontiguousarray(
        w.reshape(nchunk, 128, ncol).transpose(1, 0, 2).reshape(128, nchunk * ncol))


def _pack_v(v):
    v = np.asarray(v, np.float32).reshape(-1)
    return np.ascontiguousarray(v.reshape(4, 128).T)


def _prep_host(inputs):
    f8 = np.float64
    import ml_dtypes
    bf = ml_dtypes.bfloat16
    Wk = np.asarray(inputs["Wk"], f8)
    q_tom = np.asarray(inputs["q_tom"], f8)
    qk = np.einsum("dhk,hk->dh", Wk.reshape(D, NH, DH), q_tom) / np.sqrt(
        np.float64(DH))
    qk_pk = np.zeros((128, NCD, 16), np.float32)
    qk_pk[:, :, 0:8] = qk.astype(np.float32).reshape(NCD, 128, 8).transpose(1, 0, 2)
    qk_pk = np.ascontiguousarray(qk_pk.reshape(128, NCD * 16)).astype(bf)

    m = np.asarray(inputs["attention_mask"], f8)  # [B, S]
    cum = np.cumsum(m, axis=1)
    valid = cum[:, -1:]
    split = np.maximum(1.0, np.floor(valid * 0.6))
    setup = m * (cum <= split)
    punch = m * (cum > split)
    pc = punch.sum(1, keepdims=True)
    last = m * (cum == valid)
    punch = np.where(pc > 0, punch, last)

    cnts = np.stack([m.sum(1), setup.sum(1), punch.sum(1)], 1)  # [B, 3]
    inv_cnts = (1.0 / cnts).astype(np.float32)
    masks3 = np.stack([m, setup, punch], -1)  # [B, S, 3]

    M3 = (np.eye(3, dtype=f8)
          + np.asarray(inputs["U_mhc"], f8) @ np.asarray(inputs["V_mhc"], f8))
    m3 = np.ascontiguousarray(M3.astype(np.float32).reshape(1, 9))

    Ws1 = np.asarray(inputs["Ws1"], np.float32)
    Wf1 = np.asarray(inputs["Wf1"], np.float32)
    vecs = np.concatenate([
        _pack_v(inputs["w_hp"]), _pack_v(inputs["w_inc"]), _pack_v(inputs["wc2"]),
        _pack_v(inputs["ws2"]), _pack_v(inputs["wf2"])], axis=1)
    bvecs = np.concatenate([
        _pack_v(inputs["bg1"]), _pack_v(inputs["bg2"]), _pack_v(inputs["bc1"]),
        _pack_v(inputs["bs1"]), _pack_v(inputs["bf1"])], axis=1)
    b5 = np.array([[np.float32(np.asarray(inputs[k]).reshape(-1)[0])
                    for k in ("b_hp", "b_inc", "bc2", "bs2", "bf2")]], np.float32)

    shared = {
        "qk": qk_pk, "ident": np.eye(128, dtype=np.float32).astype(bf),
        "ident32": np.eye(16, dtype=np.float32),
        "wv": _pack_w(inputs["Wv"]).astype(bf),
        "wtf": _pack_w(inputs["W_tom_fuse"]).astype(bf),
        "wg1": _pack_w(inputs["Wg1"]).astype(bf),
        "wg2": _pack_w(inputs["Wg2"]).astype(bf),
        "wc1": _pack_w(inputs["Wc1"]).astype(bf),
        "ws1": _pack_w(Ws1[:512]).astype(bf),
        "ws1t": np.ascontiguousarray(Ws1[512:515]).astype(bf),
        "wf1": _pack_w(Wf1[:512]).astype(bf),
        "wf1t": np.ascontiguousarray(Wf1[512:515]).astype(bf),
        "vecs": np.ascontiguousarray(vecs).astype(bf),
        "bvecs": np.ascontiguousarray(bvecs),
        "b5": b5, "m3": m3,
    }
    x = np.asarray(inputs["embeddings"], np.float32)
    in_maps = []
    for k in range(NCORES):
        d = dict(shared)
        xc = x[k * BPC:(k + 1) * BPC]                  # [8, S, D]
        # xg[b, p, (g, t, d)] = x[b, g*512 + t*128 + p, d]
        xg = xc.reshape(BPC, NG, NT, 128, D).transpose(0, 3, 1, 2, 4)
        d["xg"] = np.ascontiguousarray(
            xg.reshape(BPC, 128, NG * NT * D)).astype(bf)
        # xtg[b, p, (g, c, s)] = x[b, g*512 + s, c*128 + p]
        xt = xc.transpose(0, 2, 1)                     # [8, D, S]
        xtg = xt.reshape(BPC, NCD, 128, NG, GT).transpose(0, 2, 3, 1, 4)
        d["xtg"] = np.ascontiguousarray(
            xtg.reshape(BPC, 128, NG * NCD * GT)).astype(bf)
        mc = m[k * BPC:(k + 1) * BPC]                  # [8, S]
        d["pen"] = np.ascontiguousarray(
            (-30.0 * (1.0 - mc)).reshape(1, BPC * NG * GT)).astype(
                np.float32).astype(bf)
        mk = masks3[k * BPC:(k + 1) * BPC].reshape(BPC, NG, 2, 2, 128, 3)
        maskt = mk.transpose(4, 0, 1, 2, 3, 5).reshape(128, BPC * NG * 12)
        d["maskt"] = np.ascontiguousarray(maskt).astype(np.float32).astype(bf)
        ic = np.ones((128, BPC // 2), np.float32)
        for pp in range(BPC // 2):
            for j in range(2):
                ic[32 * j + 8:32 * j + 11, pp] = inv_cnts[k * BPC + 2 * pp + j]
        d["invcnt"] = ic
        in_maps.append(d)
    return in_maps


def _install_ntff_shim():
    """antenv.axon_hooks is absent in this image; recreate it so
    run_bass_kernel_spmd(trace=True) can capture NTFF profiles."""
    import sys
    import types
    if "antenv.axon_hooks" in sys.modules:
        return
    mod = types.ModuleType("antenv.axon_hooks")
    mod._hook = None
    mod.set_axon_ntff_profile_hook = lambda h: setattr(mod, "_hook", h)
    mod.get_axon_ntff_profile_hook = lambda: mod._hook
    sys.modules["antenv.axon_hooks"] = mod
    try:
        import antenv
        antenv.axon_hooks = mod
        from trn_agent_boot.trn_boot import _ntff_profile_via_ctypes
        mod._hook = _ntff_profile_via_ctypes("/opt/axon/libaxon_pjrt.so")
    except Exception as e:
        print(f"ntff shim setup failed ({e}); tracing disabled")


def kernel(**inputs):
    global LAST_RESULT
    _install_ntff_shim()
    from concourse.bass_utils import run_bass_kernel_spmd

    if "nc" not in _CACHE:
        _CACHE["nc"] = _build_program()
    nc = _CACHE["nc"]

    in_maps = _prep_host(inputs)
    trace = os.environ.get("BASS_TRACE", "0") == "1"
    res = run_bass_kernel_spmd(nc, in_maps, list(range(NCORES)), trace=trace)
    LAST_RESULT = res
    out = np.empty((B, 1), np.float32)
    for k in range(NCORES):
        out[k * BPC:(k + 1) * BPC, 0] = np.asarray(res.results[k]["out"]).reshape(-1)
    return out

